# revision 1
# baseline (speedup 1.0000x reference)
"""Corr1d (stereo cost volume) Trainium2 kernel, v2.

corrmap[b, i, h, w] = sum_c fL[b, c, h, w] * fR[b, c, h, w - i],  i in [0, 64)
Shapes: fL, fR [8, 128, 160, 320] f32 -> corrmap [8, 64, 160, 320] f32.
Sharding: data-parallel over batch; core k handles batch element k.
Host: inputs cast f32->bf16 (RTNE) before upload; output computed in bf16 on
device and upcast f32 on host (scratch path was already bf16 -> no extra err).

Per-core pipeline (h rows processed in NB=10 batches of NH=16):
  1. Load fL/fR h-batch rows to SBUF (bf16).  fR gets 64 zero-pad columns at
     the buffer start so tile-0 reads of w-i < 0 (h row 0) hit zeros.
  2. Band matmuls, M=32 col-tiled: for each group of 4 w-tiles (32 cols
     each), 4 concurrent matmuls (tile_position=(0,32q)) share one PSUM tile
     ps[128, 2*95]: ps[32q+m', hh2*95+n] = sum_c fL[c,wt+m']*fR[c,wt+31-n]
     = corr[i = m' + n - 31, wt + m'].  Band half-width 95 (vs 191 at M=128)
     halves scratch traffic.
  3. Copy PSUM -> band tiles in h8-interleaved layout band[p, hc2*768+n*8+h8]
     (n in [0,96), h8 = h%8, hc2 = h//8 within batch).  Zero the i>w garbage
     zones (w-tiles 0/1 read the previous h row's tail for h>0) with two
     memsets after the copies.
  4. Dump bands to DRAM scratch[g, p, hc, n, h8] (rows fully contiguous,
     3KB runs), then read back realigned along the band diagonals:
     T[p, hc2*512 + i*8 + h8] = scratch[g, p, hc, 31-(p%32)+i, h8]
     = corr[i, h, w(p)] -- the per-partition skew is absorbed by flat DRAM
     addressing; (i,h8) collapse makes 1KB contiguous runs.
  5. TensorE-transpose T in [*,128] chunks (chunk = 16 i x 8 h8) ->
     U[(i16,h8), w-cols] in PSUM, copy to staging stg[(i16,h8), (hc2,a,w)],
     DMA to out[i,h,w] with a 4D AP (partitions split (i16,h8)).
  Stages are software-pipelined one batch deep so the in-order PE queue never
  stalls on the dump->readback round trip.

Self-contained: shapes hardcoded; requires only numpy + ml_dtypes + concourse.
"""

import ml_dtypes
import numpy as np

import concourse.bacc as bacc
import concourse.bass as bass
import concourse.mybir as mybir
from concourse.bass_utils import run_bass_kernel_spmd
from concourse.tile import TileContext
from concourse.masks import make_identity

F32 = mybir.dt.float32
BF16 = mybir.dt.bfloat16

N_CORES = 8
C = 128           # channels (matmul contraction dim)
H = 160
W = 320
D = 64            # disparities
NH = 16           # h rows per batch
NB = H // NH      # batches (10)
NS = 95           # band columns per 32-wide w-tile (32 + 63)
SC = 96           # stored band columns (n dim) per tile row
FRPAD = 64        # zero pad columns at the start of the fR buffer
HW = H * W
HC = H // 8       # h-chunks of 8 rows (20)
ROW = SC * 8      # 768: scratch row (n, h8) elements per (p, hc)
GP = HC * ROW     # 15360: scratch elements per partition-row
GG = 128 * GP     # scratch elements per group

_cache = {}


def _build(repeat: int = 1, stages: str = "all"):
    nc = bacc.Bacc("TRN2", target_bir_lowering=False, debug=False,
                   num_devices=N_CORES)
    fL = nc.dram_tensor("fL", [C, H, W], BF16, kind="ExternalInput")
    fR = nc.dram_tensor("fR", [C, H, W], BF16, kind="ExternalInput")
    out = nc.dram_tensor("out", [D, H, W], BF16, kind="ExternalOutput")
    scratch = nc.dram_tensor("scratch", [3, 128, HC, SC, 8], BF16)

    with TileContext(nc) as tc:
        fLb = [nc.alloc_sbuf_tensor(f"fLb{i}", [C, NH * W], BF16)
               for i in range(2)]
        fRb = [nc.alloc_sbuf_tensor(f"fRb{i}", [C, FRPAD + NH * W], BF16)
               for i in range(2)]
        ident = nc.alloc_sbuf_tensor("ident", [128, 128], BF16)
        make_identity(nc, ident.ap())
        for i in range(2):
            nc.vector.memset(fRb[i].ap()[:, 0:FRPAD], 0.0)

        from contextlib import ExitStack
        stack = ExitStack()
        if repeat > 1:
            stack.enter_context(tc.For_i(0, repeat, 1))
        with (
            stack,
            tc.tile_pool(name="sb", bufs=2) as pool,
            tc.tile_pool(name="ps", bufs=2, space="PSUM") as pp,
        ):
            def emit_loads(b):
                li, ri = fLb[b % 2], fRb[b % 2]
                h0 = b * NH
                nc.gpsimd.dma_start(
                    out=li.ap(),
                    in_=bass.AP(fL, h0 * W, [[HW, C], [1, NH * W]]),
                )
                nc.gpsimd.dma_start(
                    out=ri.ap()[:, FRPAD:],
                    in_=bass.AP(fR, h0 * W, [[HW, C], [1, NH * W]]),
                )

            def emit_front(b):
                # matmuls + psum->band copies + garbage memsets + dumps +
                # readbacks for batch b; returns the T tiles.
                li, ri = fLb[b % 2], fRb[b % 2]
                b01 = pool.tile([128, 2 * 2 * ROW], BF16, tag="b01",
                                name=f"b01_{b}")
                b2 = pool.tile([64, 2 * ROW], BF16, tag="b2", name=f"b2_{b}")
                for hq in range(NH // 4):
                    # 4 h rows share one PSUM bank (4*95 f32 = 1520B);
                    # the copy then writes 8B-coalesced (n, delta4) runs
                    # into the h8-interleaved band layout.
                    hc2, h8b = hq // 2, 4 * (hq % 2)
                    pss = []
                    for g in range(3):
                        P = 64 if g == 2 else 128
                        nt = 2 if g == 2 else 4
                        ps = pp.tile([P, 4 * NS], F32, tag=f"ps{g}",
                                     name=f"ps{g}_{b}_{hq}")
                        pss.append(ps)
                        for j4 in range(4):
                            hh = 4 * hq + j4
                            for q in range(nt):
                                wt = g * 128 + 32 * q
                                lhsT = bass.AP(li, hh * W + wt,
                                               [[NH * W, C], [1, 32]])
                                rhs = bass.AP(ri, FRPAD + hh * W + wt + 31,
                                              [[FRPAD + NH * W, C], [-1, NS]])
                                nc.tensor.matmul(
                                    ps[32 * q:32 * q + 32,
                                       j4 * NS:(j4 + 1) * NS],
                                    lhsT, rhs, start=True, stop=True,
                                    tile_position=(0, 32 * q),
                                )
                    for g in range(3):
                        P = 64 if g == 2 else 128
                        tile = b2 if g == 2 else b01
                        base = (g % 2) * 2 * ROW + hc2 * ROW + h8b
                        pitch = tile.tensor.shape[-1]
                        o = bass.AP(tile.tensor, base,
                                    [[pitch, P], [8, NS], [1, 4]])
                        i_ = bass.AP(pss[g].tensor, 0,
                                     [[4 * NS, P], [1, NS], [NS, 4]])
                        if (hq + g) % 2 == 0:
                            nc.vector.tensor_copy(out=o, in_=i_)
                        else:
                            nc.scalar.copy(o, i_)
                # zero i > w zones (w-tiles 0 and 1): band cols n>=32 / n>=64
                nc.vector.memset(
                    bass.AP(b01.tensor, 32 * 8,
                            [[4 * ROW, 32], [ROW, 2], [1, (SC - 32) * 8]]),
                    0.0)
                nc.vector.memset(
                    bass.AP(b01.tensor, 32 * (4 * ROW) + 64 * 8,
                            [[4 * ROW, 32], [ROW, 2], [1, (SC - 64) * 8]]),
                    0.0)

                if stages == "front":
                    nc.sync.dma_start(
                        out=bass.AP(out, 0, [[W, 64], [1, 64]]),
                        in_=b01[:64, 0:64],
                    )
                    return None
                # dumps (2 hc chunks land at hc = 2b, 2b+1)
                nc.sync.dma_start(
                    out=bass.AP(scratch, 2 * b * ROW,
                                [[GP, 128], [GG, 2], [1, 2 * ROW]]),
                    in_=b01[:, :],
                )
                nc.sync.dma_start(
                    out=bass.AP(scratch, 2 * GG + 2 * b * ROW,
                                [[GP, 64], [1, 2 * ROW]]),
                    in_=b2[:, :],
                )
                # diagonal readbacks: T[p, hc2*512 + i*8 + h8]
                # (per (g, hc2) to stay within the 3-dim AP-balance limit;
                # SWDGE queue keeps the issue cost off the sync engine)
                Ts = []
                for g in range(3):
                    P = 64 if g == 2 else 128
                    T = pool.tile([P, 2 * 512], BF16, tag=f"T{g}",
                                  name=f"T{g}_{b}")
                    Ts.append(T)
                    for hc2 in range(2):
                        # sync queue: HWDGE FIFO orders these after the
                        # dumps (DRAM deps are not tile-tracked)
                        nc.sync.dma_start(
                            out=T[:, hc2 * 512:(hc2 + 1) * 512],
                            in_=bass.AP(scratch,
                                        g * GG + (2 * b + hc2) * ROW + 31 * 8,
                                        [[32 * GP, P // 32], [GP - 8, 32],
                                         [1, 512]]),
                        )
                if stages == "mid":
                    for g in range(3):
                        nc.sync.dma_start(
                            out=bass.AP(out, 0, [[W, 64], [1, 64]]),
                            in_=Ts[g][:64, 0:64],
                        )
                    return None
                return Ts

            def emit_back(b, Ts):
                # transposes + staging copies + output DMAs for batch b
                stg = pool.tile([128, 2 * 4 * W], BF16, tag="stg",
                                name=f"stg_{b}")
                for hc2 in range(2):
                    for a in range(4):
                        u = pp.tile([128, W], BF16, tag="U",
                                    name=f"U_{b}_{hc2}_{a}")
                        cs = hc2 * 512 + 128 * a
                        nc.tensor.transpose(
                            u[:, 0:128], Ts[0][:, cs:cs + 128], ident.ap())
                        nc.tensor.transpose(
                            u[:, 128:256], Ts[1][:, cs:cs + 128], ident.ap())
                        nc.tensor.transpose(
                            u[:, 256:320], Ts[2][:, cs:cs + 128],
                            ident.ap()[0:64, 0:64])
                        o = stg[:, (hc2 * 4 + a) * W:(hc2 * 4 + a + 1) * W]
                        if a % 2 == 0:
                            nc.vector.tensor_copy(out=o, in_=u[:, :])
                        else:
                            nc.scalar.copy(o, u[:, :])
                for hc2 in range(2):
                    for a in range(4):
                        eng = nc.scalar if a % 2 == 0 else nc.sync
                        eng.dma_start(
                            out=bass.AP(out,
                                        16 * a * HW + (16 * b + 8 * hc2) * W,
                                        [[HW, 16], [W, 8], [1, W]]),
                            in_=bass.AP(stg.tensor, (hc2 * 4 + a) * W,
                                        [[8 * W, 128], [1, W]]),
                        )

            # software pipeline: loads one batch ahead, back-stages one behind
            emit_loads(0)
            prev = None
            for b in range(NB):
                if b + 1 < NB:
                    emit_loads(b + 1)
                if prev is not None:
                    emit_back(b - 1, prev)
                prev = emit_front(b)
                if prev is None and stages != "all":
                    continue
            if prev is not None:
                emit_back(NB - 1, prev)

    nc.compile()
    return nc


def _make_in_maps(inputs: dict) -> list:
    fL = np.asarray(inputs["fL"], dtype=np.float32).astype(ml_dtypes.bfloat16)
    fR = np.asarray(inputs["fR"], dtype=np.float32).astype(ml_dtypes.bfloat16)
    fL = np.ascontiguousarray(fL)
    fR = np.ascontiguousarray(fR)
    return [{"fL": fL[k], "fR": fR[k]} for k in range(N_CORES)]


def kernel(fL: np.ndarray, fR: np.ndarray) -> np.ndarray:
    if "nc" not in _cache:
        _cache["nc"] = _build()
    nc = _cache["nc"]

    in_maps = _make_in_maps({"fL": fL, "fR": fR})
    res = run_bass_kernel_spmd(nc, in_maps, core_ids=list(range(N_CORES)))
    out = np.stack(
        [res.results[k]["out"].astype(np.float32) for k in range(N_CORES)],
        axis=0,
    )
    return out


if __name__ == "__main__":
    rng = np.random.default_rng(0)
    a = rng.standard_normal((N_CORES, C, H, W)).astype(np.float32)
    b = rng.standard_normal((N_CORES, C, H, W)).astype(np.float32)
    o = kernel(a, b)
    print("kernel ran, output shape", o.shape)

